# revision 39
# baseline (speedup 1.0000x reference)
"""Two-layer GCN block (GCNConv + LayerNorm + ELU, x2) on 8 Trainium2 NeuronCores.

Strategy
--------
Nodes are padded to a multiple of 128*8 and dealt round-robin (block of 128
nodes at a time) to the 8 cores. Per layer, each core:
  1. computes hs = (act @ W) * dinv (bf16) for its own node shard on the PE
     and seeds a persistent accumulator with the local self-loop term,
  2. AllGathers the shard rows into two replicated bf16 DRAM tables
     (A: <=32768 rows, B: the rest) so row ids fit dma_gather's int16 index;
     AG-A is issued as soon as the A-range rows are staged, AG-B between the
     two conv passes so both transfers hide under compute/gather work,
  3. pass A: per 128-node iteration, dma_gathers the 256B bf16 rows of all
     table-A in-edge sources (exact per-iteration slot count, split across
     4 SWDGE queues), slot-reduces on the vector engine into the accumulator;
     pass B: same for table-B sources, then batched dest-norm + LayerNorm +
     ELU over groups of iterations.
Symmetric normalization deg^-1/2[src]*deg^-1/2[dst] is applied as a row scale
of hs (source side) and after the slot reduce (dest side); self edges never
hit the gather path. Slot padding points at an all-zero dummy row (dinv = 0),
so pads contribute exactly zero. Nodes are clustered into iterations by
(#A-sources, snake #B-sources) so per-iteration slot counts are near-exact.

The kernel is dma_gather descriptor-bound (~5-10 ns per gathered row), so
everything else (PE matmuls, DVE reduces/LN, both AllGathers) hides under
the gather streams; minimizing gathered rows is the main lever.

All graph preprocessing (degree, permutation, slot index arrays) is host-side
numpy on edge_index only; all floating point work on x happens on-device.
"""

import math

import numpy as np

import concourse.bacc as bacc
import concourse.bass_utils as bass_utils
import concourse.mybir as mybir
import concourse.tile as tile
from concourse.masks import make_identity

P = 128
NC = 8
LN_EPS = 1e-5
_ACAP = 32768  # max rows of gather table A (int16 index limit)
F32 = mybir.dt.float32
BF16 = mybir.dt.bfloat16
I16 = mybir.dt.int16
AF = mybir.ActivationFunctionType
ALU = mybir.AluOpType


# ----------------------------------------------------------------------------
# Host-side graph preprocessing (depends only on edge_index + shapes)
# ----------------------------------------------------------------------------

def _preprocess(edge_index, n_nodes):
    N = n_nodes
    src = np.asarray(edge_index[0], dtype=np.int64)
    dst = np.asarray(edge_index[1], dtype=np.int64)

    # degree INCLUDES the self-loop (reference adds one per node), but the
    # self contribution is computed locally on-device (acc init = mm*dinv^2),
    # so self edges are excluded from the gather slot lists.
    deg = (np.bincount(dst, minlength=N) + 1).astype(np.float64)
    dinv = (1.0 / np.sqrt(deg)).astype(np.float32)

    bpc = math.ceil(N / (P * NC))
    if bpc * P * NC - N < 2:
        bpc += 1
    NPAD = bpc * P * NC
    ndum = NPAD - N

    nbA = min(bpc, _ACAP // (P * NC))
    nbB = bpc - nbA
    rowsA = NC * nbA * P
    rowsB = NC * nbB * P
    assert rowsA <= 32768 and rowsB <= 32768

    # --- assign nodes to table A / table B -------------------------------
    # A gets the (rowsA - 1) highest-degree real nodes + one dummy;
    # B gets everything else. Dummy node ids are N..NPAD-1 (deg 0, dinv 0).
    order_deg = np.argsort(-deg, kind="stable")
    if nbB > 0:
        a_real = order_deg[: rowsA - 1]
        b_real = order_deg[rowsA - 1:]
        a_nodes = np.concatenate([a_real, [N]])               # one dummy in A
        b_nodes = np.concatenate([b_real, np.arange(N + 1, NPAD)])
    else:
        a_nodes = np.concatenate([order_deg, np.arange(N, NPAD)])
        b_nodes = np.arange(0)
    assert len(a_nodes) == rowsA and len(b_nodes) == rowsB

    in_a = np.zeros(NPAD, dtype=bool)
    in_a[a_nodes] = True

    # --- per-node source-table counts -----------------------------------
    src_in_a = in_a[src]
    l_cnt = np.bincount(dst[src_in_a], minlength=N).astype(np.int64)
    h_cnt = np.bincount(dst, minlength=N).astype(np.int64) - l_cnt
    l_pad = np.zeros(NPAD, dtype=np.int64)
    h_pad = np.zeros(NPAD, dtype=np.int64)
    l_pad[:N] = l_cnt
    h_pad[:N] = h_cnt

    # --- cluster nodes into blocks by (l, snake-h) ----------------------
    def place(nodes, nb):
        """Sort nodes by (l, snake h), chunk into blocks of P, deal blocks
        round-robin to cores grouped by iteration. Returns (node_at, id_of)
        where node_at[c, t_local, p] = node id and id_of[node] = table-local
        row id c*(nb*P) + t_local*P + p."""
        ln = l_pad[nodes]
        hn = h_pad[nodes]
        snake = np.where(ln % 2 == 0, hn, hn.max() + 1 - hn)
        order = np.lexsort((snake, ln))
        snodes = nodes[order]
        nblk = len(nodes) // P
        assert nblk == nb * NC
        node_at = np.empty((NC, nb, P), dtype=np.int64)
        id_of = np.empty(NPAD, dtype=np.int64)
        q = np.arange(len(nodes))
        blk = q // P
        t_loc = blk // NC
        core = blk % NC
        pos = q % P
        node_at[core, t_loc, pos] = snodes
        id_of[snodes] = core * (nb * P) + t_loc * P + pos
        return node_at, id_of

    a_at, a_id = place(a_nodes, nbA)
    if nbB > 0:
        b_at, b_id = place(b_nodes, nbB)
    else:
        b_at = np.empty((NC, 0, P), dtype=np.int64)
        b_id = np.zeros(NPAD, dtype=np.int64)

    # node -> (core, iter t in [0, bpc), partition p)
    node_core = np.empty(NPAD, dtype=np.int64)
    node_iter = np.empty(NPAD, dtype=np.int64)
    node_pos = np.empty(NPAD, dtype=np.int64)
    tid = a_id[a_nodes]
    node_core[a_nodes] = tid // (nbA * P)
    node_iter[a_nodes] = (tid % (nbA * P)) // P
    node_pos[a_nodes] = tid % P
    if nbB > 0:
        tid = b_id[b_nodes]
        node_core[b_nodes] = tid // (nbB * P)
        node_iter[b_nodes] = nbA + (tid % (nbB * P)) // P
        node_pos[b_nodes] = tid % P

    # node_map[c, t*P + p] = node id
    node_map = np.empty((NC, bpc * P), dtype=np.int64)
    flat = node_iter * P + node_pos
    node_map[node_core, flat] = np.arange(NPAD)

    # --- per-iteration slot counts (shared across cores) ----------------
    Kl = np.zeros(bpc, dtype=np.int64)
    Kh = np.zeros(bpc, dtype=np.int64)
    np.maximum.at(Kl, node_iter[:N], l_pad[:N])
    np.maximum.at(Kh, node_iter[:N], h_pad[:N])

    # --- group iterations into compute batches ---------------------------
    # Gathers/slot-reduces run per iteration (exact Kl[t]/Kh[t], no padding
    # to a batch max); LN/ELU/matmul run per batch of G iterations. CAP
    # bounds the summed gather width so per-batch SBUF tiles stay small.
    CAP, GMAX = _CAP, 4
    batches = []  # (t0, G)
    for lo, hi in (((0, nbA)), ((nbA, bpc))):
        t = lo
        while t < hi:
            G = 1
            while (t + G < hi and G < GMAX and
                   int(np.sum(Kl[t:t + G + 1]) + np.sum(Kh[t:t + G + 1]))
                   <= CAP):
                G += 1
            batches.append((t, G))
            t += G

    # --- slot assignment for every edge ---------------------------------
    e_dst = dst
    e_c = node_core[e_dst]
    e_t = node_iter[e_dst]
    e_p = node_pos[e_dst]
    e_f = (~src_in_a).astype(np.int64)          # 0 = A-table, 1 = B-table
    e_val = np.where(src_in_a, a_id[src], b_id[src])

    key = e_dst * 2 + e_f
    order = np.argsort(key, kind="stable")
    sk = key[order]
    starts = np.concatenate([[0], np.flatnonzero(np.diff(sk)) + 1])
    counts = np.diff(np.concatenate([starts, [len(sk)]]))
    k_in = np.arange(len(sk)) - np.repeat(starts, counts)

    cumKl = np.concatenate([[0], np.cumsum(Kl)])
    cumKh = np.concatenate([[0], np.cumsum(Kh)])
    LA = int(cumKl[-1]) * P
    LB = int(cumKh[-1]) * P

    pad_a = int(a_id[N])                         # the A dummy row
    pad_b = int(b_id[N + 1]) if nbB > 0 else 0
    valA = np.full((NC, max(LA, 1)), pad_a, dtype=np.int64)
    valB = np.full((NC, max(LB, 1)), pad_b, dtype=np.int64)

    oc = e_c[order]
    ot = e_t[order]
    op = e_p[order]
    of = e_f[order]
    ov = e_val[order]
    mA = of == 0
    posA = (cumKl[ot[mA]] + k_in[mA]) * P + op[mA]
    valA[oc[mA], posA] = ov[mA]
    if nbB > 0:
        mB = ~mA
        posB = (cumKh[ot[mB]] + k_in[mB]) * P + op[mB]
        valB[oc[mB], posB] = ov[mB]

    assert valA.max() < 32768 and valB.max() < 32768

    def wrap(vals, cum, Ks):
        """Per-iteration segments -> int16 [P, total/16] in dma_gather's
        wrapped-16 layout, replicated across the 8 partition groups."""
        cols = []
        for t in range(bpc):
            seg = vals[:, cum[t] * P:(cum[t] + Ks[t]) * P]       # [NC, Kt*P]
            if Ks[t] == 0:
                continue
            w = seg.reshape(NC, -1, 16).transpose(0, 2, 1)        # [NC, 16, Kt*8]
            cols.append(w)
        if not cols:
            return np.zeros((NC, P, 1), dtype=np.int16)
        out = np.concatenate(cols, axis=2).astype(np.int16)       # [NC, 16, CA]
        return np.tile(out, (1, 8, 1))                            # [NC, 128, CA]

    idxA = wrap(valA, cumKl, Kl)
    idxB = wrap(valB, cumKh, Kh) if nbB > 0 else np.zeros((NC, P, 1), np.int16)

    dinv_pad = np.zeros(NPAD, dtype=np.float32)
    dinv_pad[:N] = dinv

    return dict(
        N=N, bpc=bpc, NPAD=NPAD, nbA=nbA, nbB=nbB, rowsA=rowsA, rowsB=rowsB,
        Kl=Kl, Kh=Kh, batches=batches, node_map=node_map,
        dinv_pad=dinv_pad,
        idxA=idxA, idxB=idxB,
        CA=idxA.shape[2], CB=idxB.shape[2],
    )


# ----------------------------------------------------------------------------
# Bass kernel builder (one NEFF, SPMD across 8 cores)
# ----------------------------------------------------------------------------

def _pieces(k):
    if _PIECE:
        n = max(1, -(-k // _PIECE))
        cuts = [k * i // n for i in range(n + 1)]
        return tuple((cuts[i], cuts[i + 1]) for i in range(n))
    if _SPLIT and k >= 16:
        return ((0, k // 2), (k // 2, k))
    return ((0, k),)


def _build_nc(meta, flags, debug_dumps=False, stop=99, reps=1):
    bpc, nbA, nbB = meta["bpc"], meta["nbA"], meta["nbB"]
    rowsA, rowsB = meta["rowsA"], meta["rowsB"]
    Kl, Kh = meta["Kl"], meta["Kh"]
    CA, CB = meta["CA"], meta["CB"]
    batches = meta["batches"]
    D = P

    nc = bacc.Bacc("TRN2", target_bir_lowering=False, debug=False,
                   num_devices=NC, num_swdge_queues=4)
    qctr = [0]

    def next_q():
        q = qctr[0] % 4
        qctr[0] += 1
        return q

    t_xT = nc.dram_tensor("xT", [P, bpc * P], F32, kind="ExternalInput")
    t_w1 = nc.dram_tensor("w1", [P, P], F32, kind="ExternalInput")
    t_w2 = nc.dram_tensor("w2", [P, P], F32, kind="ExternalInput")
    t_lnp = nc.dram_tensor("lnp", [P, 6 * P], F32, kind="ExternalInput")
    t_dinv = nc.dram_tensor("dinvb", [P, bpc], F32, kind="ExternalInput")
    t_idxA = nc.dram_tensor("idxA", [P, CA], I16, kind="ExternalInput")
    t_idxB = nc.dram_tensor("idxB", [P, CB], I16, kind="ExternalInput")
    t_out = nc.dram_tensor("out", [bpc * P, P], F32, kind="ExternalOutput")

    ag_in = {}
    ag_out = {}
    for lyr in (1, 2):
        ag_in[lyr, "A"] = nc.dram_tensor(f"agA{lyr}_in", [nbA * P, P], BF16,
                                         kind="Internal")
        ag_out[lyr, "A"] = nc.dram_tensor(f"agA{lyr}_out", [rowsA, P], BF16,
                                          kind="Internal", addr_space="Shared")
        if nbB > 0:
            ag_in[lyr, "B"] = nc.dram_tensor(f"agB{lyr}_in", [nbB * P, P],
                                             BF16, kind="Internal")
            ag_out[lyr, "B"] = nc.dram_tensor(f"agB{lyr}_out", [rowsB, P],
                                              BF16, kind="Internal",
                                              addr_space="Shared")

    cumKl = np.concatenate([[0], np.cumsum(Kl)]).astype(int)
    cumKh = np.concatenate([[0], np.cumsum(Kh)]).astype(int)
    ga_max = int(Kl.max())
    gb_max = int(Kh.max())
    bw_max = max(g for _, g in batches) * P
    # index of the last batch whose staging rows land in table A
    last_a_batch = max((i for i, (t0, _) in enumerate(batches) if t0 < nbA),
                      default=-1)

    with tile.TileContext(nc) as tc:
        with tc.tile_pool(name="const", bufs=1) as cpool, \
             tc.tile_pool(name="sb", bufs=2) as sb, \
             tc.tile_pool(name="gat", bufs=_GATBUFS) as gat, \
             tc.tile_pool(name="ixp", bufs=_GATBUFS) as ixp, \
             tc.tile_pool(name="ps", bufs=2, space="PSUM") as ps, \
             tc.tile_pool(name="ps2", bufs=2, space="PSUM") as ps2:

            w1_t = cpool.tile([P, P], F32)
            w2_t = cpool.tile([P, P], F32)
            need_lnp = any(flags.values())
            lnp_t = cpool.tile([P, 6 * P], F32) if need_lnp else None
            dinv_t = cpool.tile([P, bpc], F32)
            ident = cpool.tile([P, P], F32)
            nc.sync.dma_start(w1_t[:], t_w1[:])
            nc.sync.dma_start(w2_t[:], t_w2[:])
            if need_lnp:
                nc.sync.dma_start(lnp_t[:], t_lnp[:])
            nc.sync.dma_start(dinv_t[:], t_dinv[:])
            make_identity(nc, ident[:])

            def stage(lyr, t0, G):
                tab = "A" if t0 < nbA else "B"
                r0 = t0 * P if tab == "A" else (t0 - nbA) * P
                rows = ag_in[lyr, tab][r0:r0 + G * P, :]
                return rows.rearrange("(b p) d -> p b d", b=G)

            def allgather(lyr, tab):
                nc.gpsimd.collective_compute(
                    "AllGather", ALU.bypass,
                    replica_groups=[list(range(NC))],
                    ins=[ag_in[lyr, tab][:]],
                    outs=[ag_out[lyr, tab][:]],
                )

            # persistent per-layer partial sums (self-loop + table-A terms).
            # pass-B applies the dest-side dinv at the end, so the self term
            # seeded here carries only the source-side factor.
            acc = cpool.tile([P, bpc * P], BF16, name="acc")

            def conv_pass_a(t0, G, lyr, acc):
                """Per-iteration table-A gathers (exact Kl[t] slots each),
                reduced and added into the persistent accumulator columns
                (initialized with the local self-loop term in the mm phase).
                Depends on AG-A only."""
                for t in range(t0, t0 + G):
                    kl = int(Kl[t])
                    if kl == 0:
                        continue
                    w = kl * P
                    g = gat.tile([P, max(ga_max, 1) * P], BF16, tag="ga")
                    ixa = ixp.tile([P, max(ga_max, 1) * 8], I16,
                                   tag="ixa", name="ixa")[:, :kl * 8]
                    nc.sync.dma_start(
                        ixa, t_idxA[:, cumKl[t] * 8:cumKl[t + 1] * 8])
                    pieces = _pieces(kl)
                    for s0, s1 in pieces:
                        nc.gpsimd.dma_gather(
                            out_ap=g[:, s0 * P:s1 * P].rearrange(
                                "p (k d) -> p k d", d=P),
                            in_ap=ag_out[lyr, "A"][:],
                            idxs_ap=ixa[:, s0 * 8:s1 * 8],
                            num_idxs=(s1 - s0) * P, num_idxs_reg=(s1 - s0) * P,
                            elem_size=P, single_packet=False,
                            queue_num=next_q())
                    ra = sb.tile([P, P], F32, tag="ra", name="ra")
                    nc.vector.tensor_reduce(
                        out=ra[:],
                        in_=g[:, :w].rearrange("p (k d) -> p d k", d=P),
                        axis=mybir.AxisListType.X, op=ALU.add)
                    rc = sb.tile([P, P], BF16, tag="rc", name="rc")
                    nc.vector.tensor_copy(rc, ra)
                    av = acc[:, t * P:(t + 1) * P]
                    nc.vector.tensor_tensor(out=av, in0=av, in1=rc,
                                            op=ALU.add)

            def conv_pass_b(t0, G, lyr, acc):
                """Per-iteration table-B gathers, combined with the
                accumulator, then batched dest-norm + LayerNorm + ELU.
                Returns t1 = elu + 1. Depends on AG-B."""
                dv = dinv_t[:, t0:t0 + G].to_broadcast((P, G, P))
                red = sb.tile([P, bw_max], F32, tag="red", name="red")[:, :G * P]
                rv = red.rearrange("p (b d) -> p b d", b=G)
                av = acc[:, t0 * P:(t0 + G) * P].rearrange(
                    "p (b d) -> p b d", b=G)
                redb = sb.tile([P, bw_max], F32, tag="redb",
                               name="redb")[:, :G * P]
                any_b = False
                for t in range(t0, t0 + G):
                    kh = int(Kh[t])
                    col = redb[:, (t - t0) * P:(t - t0 + 1) * P]
                    if kh == 0:
                        nc.vector.memset(col, 0.0)
                        continue
                    any_b = True
                    w = kh * P
                    g = gat.tile([P, max(gb_max, 1) * P], BF16, tag="gb")
                    ixb = ixp.tile([P, max(gb_max, 1) * 8], I16,
                                   tag="ixb", name="ixb")[:, :kh * 8]
                    nc.sync.dma_start(
                        ixb, t_idxB[:, cumKh[t] * 8:cumKh[t + 1] * 8])
                    pieces = _pieces(kh)
                    for s0, s1 in pieces:
                        nc.gpsimd.dma_gather(
                            out_ap=g[:, s0 * P:s1 * P].rearrange(
                                "p (k d) -> p k d", d=P),
                            in_ap=ag_out[lyr, "B"][:],
                            idxs_ap=ixb[:, s0 * 8:s1 * 8],
                            num_idxs=(s1 - s0) * P, num_idxs_reg=(s1 - s0) * P,
                            elem_size=P, single_packet=False,
                            queue_num=next_q())
                    nc.vector.tensor_reduce(
                        out=col,
                        in_=g[:, :w].rearrange("p (k d) -> p d k", d=P),
                        axis=mybir.AxisListType.X, op=ALU.add)
                nc.vector.tensor_copy(red, acc[:, t0 * P:(t0 + G) * P])
                if any_b:
                    nc.vector.tensor_tensor(
                        out=rv, in0=rv,
                        in1=redb.rearrange("p (b d) -> p b d", b=G),
                        op=ALU.add)
                # dest-side norm
                nc.vector.tensor_tensor(out=rv, in0=rv, in1=dv,
                                        op=ALU.mult)
                brow = 0 if lyr == 1 else 3
                if flags[f"use_b{lyr}"]:
                    bc = lnp_t[:, brow * P:(brow + 1) * P].rearrange(
                        "p d -> p () d").to_broadcast((P, G, P))
                    nc.vector.tensor_tensor(out=rv, in0=rv, in1=bc, op=ALU.add)
                # LayerNorm (stats per node = per (partition, block))
                s = sb.tile([P, bw_max // P], F32, tag="s", name="s")[:, :G]
                nc.vector.tensor_reduce(out=s, in_=rv,
                                        axis=mybir.AxisListType.X, op=ALU.add)
                nmu = sb.tile([P, bw_max // P], F32, tag="nmu", name="nmu")[:, :G]
                nc.vector.tensor_scalar_mul(nmu, s, -1.0 / D)
                cen = sb.tile([P, bw_max], F32, tag="cen", name="cen")[:, :G * P]
                cv = cen.rearrange("p (b d) -> p b d", b=G)
                nc.vector.tensor_tensor(
                    out=cv, in0=rv,
                    in1=nmu.rearrange("p b -> p b ()").to_broadcast((P, G, P)),
                    op=ALU.add)
                sq = sb.tile([P, bw_max], F32, tag="sq", name="sq")[:, :G * P]
                nc.scalar.activation(out=sq, in_=cen, func=AF.Square)
                vs = sb.tile([P, bw_max // P], F32, tag="vs", name="vs")[:, :G]
                nc.vector.tensor_reduce(
                    out=vs, in_=sq.rearrange("p (b d) -> p b d", b=G),
                    axis=mybir.AxisListType.X, op=ALU.add)
                var = sb.tile([P, bw_max // P], F32, tag="var", name="var")[:, :G]
                nc.vector.tensor_scalar(out=var, in0=vs, scalar1=1.0 / D,
                                        scalar2=LN_EPS, op0=ALU.mult,
                                        op1=ALU.add)
                rec = sb.tile([P, bw_max // P], F32, tag="rec", name="rec")[:, :G]
                nc.vector.reciprocal(rec, var)
                rstd = sb.tile([P, bw_max // P], F32, tag="rstd", name="rstd")[:, :G]
                nc.scalar.activation(out=rstd, in_=rec, func=AF.Sqrt)
                y = cen
                yv = cv
                nc.vector.tensor_tensor(
                    out=yv, in0=cv,
                    in1=rstd.rearrange("p b -> p b ()").to_broadcast((P, G, P)),
                    op=ALU.mult)
                grow = 1 if lyr == 1 else 4
                berow = 2 if lyr == 1 else 5
                if flags[f"use_g{lyr}"]:
                    bc = lnp_t[:, grow * P:(grow + 1) * P].rearrange(
                        "p d -> p () d").to_broadcast((P, G, P))
                    nc.vector.tensor_tensor(out=yv, in0=yv, in1=bc,
                                            op=ALU.mult)
                if flags[f"use_be{lyr}"]:
                    bc = lnp_t[:, berow * P:(berow + 1) * P].rearrange(
                        "p d -> p () d").to_broadcast((P, G, P))
                    nc.vector.tensor_tensor(out=yv, in0=yv, in1=bc,
                                            op=ALU.add)
                # ELU + 1 = relu(y) + exp(min(y, 0))
                m = sb.tile([P, bw_max], F32, tag="m", name="m")[:, :G * P]
                nc.vector.tensor_scalar_min(m, y, 0.0)
                e = sb.tile([P, bw_max], F32, tag="e", name="e")[:, :G * P]
                nc.scalar.activation(out=e, in_=m, func=AF.Exp)
                r = sb.tile([P, bw_max], F32, tag="m", name="r")[:, :G * P]
                nc.vector.tensor_scalar_max(r, y, 0.0)
                t1 = sb.tile([P, bw_max], F32, tag="t1", name="t1")[:, :G * P]
                nc.vector.tensor_tensor(out=t1, in0=r, in1=e, op=ALU.add)
                return t1

            for _rep in range(reps):
                # ------------- phase 1: hs1 = (x @ W1) * dinv ---------------
                for i, (t0, G) in (enumerate(batches) if stop >= 1 else ()):
                    xT_b = sb.tile([P, bw_max], F32, tag="xT", name="xT")[:, :G * P]
                    nc.sync.dma_start(xT_b, t_xT[:, t0 * P:(t0 + G) * P])
                    mm = ps.tile([P, bw_max], F32, tag="mm", name="mm",
                                 space="PSUM")[:, :G * P]
                    for j in range(G):
                        nc.tensor.matmul(out=mm[:, j * P:(j + 1) * P],
                                         lhsT=xT_b[:, j * P:(j + 1) * P],
                                         rhs=w1_t[:], start=(j % 4 == 0),
                                         stop=(j % 4 == 3 or j == G - 1))
                    hs = sb.tile([P, bw_max], BF16, tag="hs",
                                 name="hs")[:, :G * P]
                    nc.vector.tensor_tensor(
                        out=hs.rearrange("p (b d) -> p b d", b=G),
                        in0=mm.rearrange("p (b d) -> p b d", b=G),
                        in1=dinv_t[:, t0:t0 + G].to_broadcast((P, G, P)),
                        op=ALU.mult)
                    # local self-loop term (source-side factor only):
                    # acc = (x@W) * dinv; pass-B multiplies by dinv[dst].
                    nc.vector.tensor_tensor(
                        out=acc[:, t0 * P:(t0 + G) * P].rearrange(
                            "p (b d) -> p b d", b=G),
                        in0=mm.rearrange("p (b d) -> p b d", b=G),
                        in1=dinv_t[:, t0:t0 + G].to_broadcast((P, G, P)),
                        op=ALU.mult)
                    nc.sync.dma_start(
                        stage(1, t0, G),
                        hs.rearrange("p (b d) -> p b d", b=G))
                    if stop >= 2 and i == last_a_batch:
                        allgather(1, "A")

                # ------------- phase 2: conv1 (A pass, then B pass) ---------
                for t0, G in (batches if stop >= 3 else ()):
                    conv_pass_a(t0, G, 1, acc)
                # issue AG-B after pass-A triggers so its collective time
                # overlaps the in-flight A gathers/reduces
                if stop >= 2 and nbB > 0:
                    allgather(1, "B")
                for i, (t0, G) in (enumerate(batches) if stop >= 3 else ()):
                    t1 = conv_pass_b(t0, G, 1, acc)
                    if stop == 3:
                        nc.sync.dma_start(
                            t_out[t0 * P:(t0 + G) * P, :].rearrange(
                                "(b p) d -> p b d", b=G),
                            t1.rearrange("p (b d) -> p b d", b=G))
                        continue
                    nc.vector.tensor_scalar_add(t1, t1, -1.0)
                    z = t1
                    nc.vector.tensor_tensor(
                        out=z.rearrange("p (b d) -> p b d", b=G),
                        in0=z.rearrange("p (b d) -> p b d", b=G),
                        in1=dinv_t[:, t0:t0 + G].to_broadcast((P, G, P)),
                        op=ALU.mult)
                    zT_ps = ps2.tile([P, bw_max], F32, tag="zT", name="zT",
                                     space="PSUM")[:, :G * P]
                    for j in range(G):
                        nc.tensor.matmul(out=zT_ps[:, j * P:(j + 1) * P],
                                         lhsT=z[:, j * P:(j + 1) * P],
                                         rhs=ident[:], is_transpose=True,
                                         start=(j % 4 == 0),
                                         stop=(j % 4 == 3 or j == G - 1))
                    zT = sb.tile([P, bw_max], F32, tag="zTs", name="zTs")[:, :G * P]
                    nc.vector.tensor_copy(zT, zT_ps)
                    mm2 = ps.tile([P, bw_max], F32, tag="mm", name="mm",
                                  space="PSUM")[:, :G * P]
                    for j in range(G):
                        nc.tensor.matmul(out=mm2[:, j * P:(j + 1) * P],
                                         lhsT=zT[:, j * P:(j + 1) * P],
                                         rhs=w2_t[:], start=(j % 4 == 0),
                                         stop=(j % 4 == 3 or j == G - 1))
                    hs2 = sb.tile([P, bw_max], BF16, tag="hs",
                                  name="hs")[:, :G * P]
                    nc.vector.tensor_copy(hs2, mm2)
                    hs2_f32 = mm2
                    # layer-2 self-loop term overwrites acc after pass-B
                    # read these columns (WAR handled by the tile deps).
                    # mm2 = dinv*h2 already carries the source-side factor;
                    # pass-B's dest-side dinv completes dinv^2 * h2.
                    nc.vector.tensor_copy(
                        acc[:, t0 * P:(t0 + G) * P], hs2_f32)
                    nc.sync.dma_start(
                        stage(2, t0, G),
                        hs2.rearrange("p (b d) -> p b d", b=G))
                    if stop >= 4 and i == last_a_batch:
                        allgather(2, "A")

                # ------------- phase 3: conv2 (A pass, then B pass) ---------
                for t0, G in (batches if stop >= 5 else ()):
                    conv_pass_a(t0, G, 2, acc)
                if stop >= 4 and nbB > 0:
                    allgather(2, "B")
                for t0, G in (batches if stop >= 5 else ()):
                    t1 = conv_pass_b(t0, G, 2, acc)
                    fin = t1
                    nc.vector.tensor_scalar_add(fin, t1, -1.0)
                    nc.sync.dma_start(
                        t_out[t0 * P:(t0 + G) * P, :].rearrange(
                            "(b p) d -> p b d", b=G),
                        fin.rearrange("p (b d) -> p b d", b=G))

    nc.compile()
    return nc


# ----------------------------------------------------------------------------
# Entry point
# ----------------------------------------------------------------------------

_CONV_UPTO = "full"
_SPLIT = 1  # split big per-iteration gathers across queues
_GATBUFS = 8  # in-flight gather tiles per table
_CAP = 176   # summed slot width per compute batch
_PIECE = 14  # 0: halve gathers >=16 slots; else target slots/piece

_CACHE = {}


_META_CACHE = {}


def _get_compiled(edge_index, n_nodes, flags, reps=1, stop=99):
    import hashlib
    h = hashlib.sha1(np.ascontiguousarray(edge_index).tobytes()).hexdigest()
    mkey = (h, n_nodes, _CAP)
    if mkey not in _META_CACHE:
        _META_CACHE[mkey] = _preprocess(edge_index, n_nodes)
    key = (h, n_nodes, tuple(sorted(flags.items())), reps, stop, _SPLIT, _GATBUFS, _CAP, _PIECE)
    if key not in _CACHE:
        meta = _META_CACHE[mkey]
        nc = _build_nc(meta, flags, reps=reps, stop=stop)
        _CACHE[key] = (meta, nc)
    return _CACHE[key]


def _in_maps(inputs, meta):
    """Per-core input dicts for the compiled NEFF."""
    x = np.asarray(inputs["x"], dtype=np.float32)
    N = x.shape[0]
    bpc = meta["bpc"]
    lnp = np.zeros((P, 6 * P), dtype=np.float32)
    for i, k in enumerate(("b1", "g1", "be1", "b2", "g2", "be2")):
        lnp[:, i * P:(i + 1) * P] = np.asarray(
            inputs[k], dtype=np.float32)[None, :]
    x_pad = np.zeros((meta["NPAD"], P), dtype=np.float32)
    x_pad[:N] = x
    in_maps = []
    for c in range(NC):
        nm = meta["node_map"][c]
        in_maps.append({
            "xT": np.ascontiguousarray(x_pad[nm].T),
            "w1": np.ascontiguousarray(inputs["W1"], dtype=np.float32),
            "w2": np.ascontiguousarray(inputs["W2"], dtype=np.float32),
            "lnp": lnp,
            "dinvb": np.ascontiguousarray(
                meta["dinv_pad"][nm].reshape(bpc, P).T),
            "idxA": np.ascontiguousarray(meta["idxA"][c]),
            "idxB": np.ascontiguousarray(meta["idxB"][c]),
        })
    return in_maps


def _run(x, edge_index, W1, b1, g1, be1, W2, b2, g2, be2,
         trace=False, reps=1, stop=99, **run_kwargs):
    x = np.asarray(x, dtype=np.float32)
    N, D = x.shape
    assert D == P
    flags = {
        "use_b1": bool(np.any(np.asarray(b1) != 0)),
        "use_g1": bool(np.any(np.asarray(g1) != 1)),
        "use_be1": bool(np.any(np.asarray(be1) != 0)),
        "use_b2": bool(np.any(np.asarray(b2) != 0)),
        "use_g2": bool(np.any(np.asarray(g2) != 1)),
        "use_be2": bool(np.any(np.asarray(be2) != 0)),
    }
    meta, nc = _get_compiled(np.asarray(edge_index), N, flags, reps=reps,
                             stop=stop)
    in_maps = _in_maps(
        dict(x=x, W1=W1, b1=b1, g1=g1, be1=be1, W2=W2, b2=b2, g2=g2,
             be2=be2), meta)

    res = bass_utils.run_bass_kernel_spmd(
        nc, in_maps, core_ids=list(range(NC)), trace=trace, **run_kwargs)

    out = np.empty((meta["NPAD"], P), dtype=np.float32)
    for c in range(NC):
        out[meta["node_map"][c]] = res.results[c]["out"]
    return out[:N], res


def kernel(**inputs):
    out, _ = _run(**inputs)
    return out



# revision 41
# speedup vs baseline: 1.0986x; 1.0986x over previous
"""Two-layer GCN block (GCNConv + LayerNorm + ELU, x2) on 8 Trainium2 NeuronCores.

Strategy
--------
Nodes are padded to a multiple of 128*8 and dealt round-robin (block of 128
nodes at a time) to the 8 cores. Per layer, each core:
  1. computes hs = (act @ W) * dinv (bf16) for its own node shard on the PE
     and seeds a persistent accumulator with the local self-loop term,
  2. AllGathers the shard rows into two replicated bf16 DRAM tables
     (A: <=32768 rows, B: the rest) so row ids fit dma_gather's int16 index;
     AG-A is issued as soon as the A-range rows are staged, AG-B between the
     two conv passes so both transfers hide under compute/gather work,
  3. pass A: per 128-node iteration, dma_gathers the 256B bf16 rows of all
     table-A in-edge sources (exact per-iteration slot count, split across
     4 SWDGE queues), slot-reduces on the vector engine into the accumulator;
     pass B: same for table-B sources, then batched dest-norm + LayerNorm +
     ELU over groups of iterations.
Symmetric normalization deg^-1/2[src]*deg^-1/2[dst] is applied as a row scale
of hs (source side) and after the slot reduce (dest side); self edges never
hit the gather path. Slot padding points at an all-zero dummy row (dinv = 0),
so pads contribute exactly zero. Nodes are clustered into iterations by
(#A-sources, snake #B-sources) so per-iteration slot counts are near-exact.

The kernel is dma_gather descriptor-bound (~5-10 ns per gathered row), so
everything else (PE matmuls, DVE reduces/LN, both AllGathers) hides under
the gather streams; minimizing gathered rows is the main lever.

All graph preprocessing (degree, permutation, slot index arrays) is host-side
numpy on edge_index only; all floating point work on x happens on-device.
"""

import math

import numpy as np

import concourse.bacc as bacc
import concourse.bass_utils as bass_utils
import concourse.mybir as mybir
import concourse.tile as tile
from concourse.masks import make_identity

P = 128
NC = 8
LN_EPS = 1e-5
_ACAP = 32768  # max rows of gather table A (int16 index limit)
F32 = mybir.dt.float32
BF16 = mybir.dt.bfloat16
I16 = mybir.dt.int16
AF = mybir.ActivationFunctionType
ALU = mybir.AluOpType


# ----------------------------------------------------------------------------
# Host-side graph preprocessing (depends only on edge_index + shapes)
# ----------------------------------------------------------------------------

def _preprocess(edge_index, n_nodes):
    N = n_nodes
    src = np.asarray(edge_index[0], dtype=np.int64)
    dst = np.asarray(edge_index[1], dtype=np.int64)

    # degree INCLUDES the self-loop (reference adds one per node), but the
    # self contribution is computed locally on-device (acc init = mm*dinv^2),
    # so self edges are excluded from the gather slot lists.
    deg = (np.bincount(dst, minlength=N) + 1).astype(np.float64)
    dinv = (1.0 / np.sqrt(deg)).astype(np.float32)

    bpc = math.ceil(N / (P * NC))
    if bpc * P * NC - N < 2:
        bpc += 1
    NPAD = bpc * P * NC
    ndum = NPAD - N

    nbA = min(bpc, _ACAP // (P * NC))
    nbB = bpc - nbA
    rowsA = NC * nbA * P
    rowsB = NC * nbB * P
    assert rowsA <= 32768 and rowsB <= 32768

    # --- assign nodes to table A / table B -------------------------------
    # A gets the (rowsA - 1) highest-degree real nodes + one dummy;
    # B gets everything else. Dummy node ids are N..NPAD-1 (deg 0, dinv 0).
    order_deg = np.argsort(-deg, kind="stable")
    if nbB > 0:
        a_real = order_deg[: rowsA - 1]
        b_real = order_deg[rowsA - 1:]
        a_nodes = np.concatenate([a_real, [N]])               # one dummy in A
        b_nodes = np.concatenate([b_real, np.arange(N + 1, NPAD)])
    else:
        a_nodes = np.concatenate([order_deg, np.arange(N, NPAD)])
        b_nodes = np.arange(0)
    assert len(a_nodes) == rowsA and len(b_nodes) == rowsB

    in_a = np.zeros(NPAD, dtype=bool)
    in_a[a_nodes] = True

    # --- per-node source-table counts -----------------------------------
    src_in_a = in_a[src]
    l_cnt = np.bincount(dst[src_in_a], minlength=N).astype(np.int64)
    h_cnt = np.bincount(dst, minlength=N).astype(np.int64) - l_cnt
    l_pad = np.zeros(NPAD, dtype=np.int64)
    h_pad = np.zeros(NPAD, dtype=np.int64)
    l_pad[:N] = l_cnt
    h_pad[:N] = h_cnt

    # --- cluster nodes into blocks by (l, snake-h) ----------------------
    def place(nodes, nb):
        """Sort nodes by (l, snake h), chunk into blocks of P, deal blocks
        round-robin to cores grouped by iteration. Returns (node_at, id_of)
        where node_at[c, t_local, p] = node id and id_of[node] = table-local
        row id c*(nb*P) + t_local*P + p."""
        ln = l_pad[nodes]
        hn = h_pad[nodes]
        snake = np.where(ln % 2 == 0, hn, hn.max() + 1 - hn)
        order = np.lexsort((snake, ln))
        snodes = nodes[order]
        nblk = len(nodes) // P
        assert nblk == nb * NC
        node_at = np.empty((NC, nb, P), dtype=np.int64)
        id_of = np.empty(NPAD, dtype=np.int64)
        q = np.arange(len(nodes))
        blk = q // P
        t_loc = blk // NC
        core = blk % NC
        pos = q % P
        node_at[core, t_loc, pos] = snodes
        id_of[snodes] = core * (nb * P) + t_loc * P + pos
        return node_at, id_of

    a_at, a_id = place(a_nodes, nbA)
    if nbB > 0:
        b_at, b_id = place(b_nodes, nbB)
    else:
        b_at = np.empty((NC, 0, P), dtype=np.int64)
        b_id = np.zeros(NPAD, dtype=np.int64)

    # node -> (core, iter t in [0, bpc), partition p)
    node_core = np.empty(NPAD, dtype=np.int64)
    node_iter = np.empty(NPAD, dtype=np.int64)
    node_pos = np.empty(NPAD, dtype=np.int64)
    tid = a_id[a_nodes]
    node_core[a_nodes] = tid // (nbA * P)
    node_iter[a_nodes] = (tid % (nbA * P)) // P
    node_pos[a_nodes] = tid % P
    if nbB > 0:
        tid = b_id[b_nodes]
        node_core[b_nodes] = tid // (nbB * P)
        node_iter[b_nodes] = nbA + (tid % (nbB * P)) // P
        node_pos[b_nodes] = tid % P

    # node_map[c, t*P + p] = node id
    node_map = np.empty((NC, bpc * P), dtype=np.int64)
    flat = node_iter * P + node_pos
    node_map[node_core, flat] = np.arange(NPAD)

    # --- per-iteration slot counts (shared across cores) ----------------
    Kl = np.zeros(bpc, dtype=np.int64)
    Kh = np.zeros(bpc, dtype=np.int64)
    np.maximum.at(Kl, node_iter[:N], l_pad[:N])
    np.maximum.at(Kh, node_iter[:N], h_pad[:N])

    # --- group iterations into compute batches ---------------------------
    # Gathers/slot-reduces run per iteration (exact Kl[t]/Kh[t], no padding
    # to a batch max); LN/ELU/matmul run per batch of G iterations. CAP
    # bounds the summed gather width so per-batch SBUF tiles stay small.
    CAP, GMAX = _CAP, 4
    batches = []  # (t0, G)
    for lo, hi in (((0, nbA)), ((nbA, bpc))):
        t = lo
        while t < hi:
            G = 1
            while (t + G < hi and G < GMAX and
                   int(np.sum(Kl[t:t + G + 1]) + np.sum(Kh[t:t + G + 1]))
                   <= CAP):
                G += 1
            batches.append((t, G))
            t += G

    # --- slot assignment for every edge ---------------------------------
    e_dst = dst
    e_c = node_core[e_dst]
    e_t = node_iter[e_dst]
    e_p = node_pos[e_dst]
    e_f = (~src_in_a).astype(np.int64)          # 0 = A-table, 1 = B-table
    e_val = np.where(src_in_a, a_id[src], b_id[src])

    key = e_dst * 2 + e_f
    order = np.argsort(key, kind="stable")
    sk = key[order]
    starts = np.concatenate([[0], np.flatnonzero(np.diff(sk)) + 1])
    counts = np.diff(np.concatenate([starts, [len(sk)]]))
    k_in = np.arange(len(sk)) - np.repeat(starts, counts)

    cumKl = np.concatenate([[0], np.cumsum(Kl)])
    cumKh = np.concatenate([[0], np.cumsum(Kh)])
    LA = int(cumKl[-1]) * P
    LB = int(cumKh[-1]) * P

    pad_a = int(a_id[N])                         # the A dummy row
    pad_b = int(b_id[N + 1]) if nbB > 0 else 0
    valA = np.full((NC, max(LA, 1)), pad_a, dtype=np.int64)
    valB = np.full((NC, max(LB, 1)), pad_b, dtype=np.int64)

    oc = e_c[order]
    ot = e_t[order]
    op = e_p[order]
    of = e_f[order]
    ov = e_val[order]
    mA = of == 0
    posA = (cumKl[ot[mA]] + k_in[mA]) * P + op[mA]
    valA[oc[mA], posA] = ov[mA]
    if nbB > 0:
        mB = ~mA
        posB = (cumKh[ot[mB]] + k_in[mB]) * P + op[mB]
        valB[oc[mB], posB] = ov[mB]

    assert valA.max() < 32768 and valB.max() < 32768

    def wrap(vals, cum, Ks):
        """Per-iteration segments -> int16 [P, total/16] in dma_gather's
        wrapped-16 layout, replicated across the 8 partition groups."""
        cols = []
        for t in range(bpc):
            seg = vals[:, cum[t] * P:(cum[t] + Ks[t]) * P]       # [NC, Kt*P]
            if Ks[t] == 0:
                continue
            w = seg.reshape(NC, -1, 16).transpose(0, 2, 1)        # [NC, 16, Kt*8]
            cols.append(w)
        if not cols:
            return np.zeros((NC, P, 1), dtype=np.int16)
        out = np.concatenate(cols, axis=2).astype(np.int16)       # [NC, 16, CA]
        return np.tile(out, (1, 8, 1))                            # [NC, 128, CA]

    idxA = wrap(valA, cumKl, Kl)
    idxB = wrap(valB, cumKh, Kh) if nbB > 0 else np.zeros((NC, P, 1), np.int16)

    dinv_pad = np.zeros(NPAD, dtype=np.float32)
    dinv_pad[:N] = dinv

    return dict(
        N=N, bpc=bpc, NPAD=NPAD, nbA=nbA, nbB=nbB, rowsA=rowsA, rowsB=rowsB,
        Kl=Kl, Kh=Kh, batches=batches, node_map=node_map,
        dinv_pad=dinv_pad,
        idxA=idxA, idxB=idxB,
        CA=idxA.shape[2], CB=idxB.shape[2],
    )


# ----------------------------------------------------------------------------
# Bass kernel builder (one NEFF, SPMD across 8 cores)
# ----------------------------------------------------------------------------

def _pieces(k):
    if _PIECE:
        n = max(1, -(-k // _PIECE))
        cuts = [k * i // n for i in range(n + 1)]
        return tuple((cuts[i], cuts[i + 1]) for i in range(n))
    if _SPLIT and k >= 16:
        return ((0, k // 2), (k // 2, k))
    return ((0, k),)


def _build_nc(meta, flags, debug_dumps=False, stop=99, reps=1):
    bpc, nbA, nbB = meta["bpc"], meta["nbA"], meta["nbB"]
    rowsA, rowsB = meta["rowsA"], meta["rowsB"]
    Kl, Kh = meta["Kl"], meta["Kh"]
    CA, CB = meta["CA"], meta["CB"]
    batches = meta["batches"]
    D = P

    nc = bacc.Bacc("TRN2", target_bir_lowering=False, debug=False,
                   num_devices=NC, num_swdge_queues=4)
    qctr = [0]

    def next_q():
        q = qctr[0] % 4
        qctr[0] += 1
        return q

    t_xT = nc.dram_tensor("xT", [P, bpc * P], F32, kind="ExternalInput")
    t_w1 = nc.dram_tensor("w1", [P, P], F32, kind="ExternalInput")
    t_w2 = nc.dram_tensor("w2", [P, P], F32, kind="ExternalInput")
    t_lnp = nc.dram_tensor("lnp", [P, 6 * P], F32, kind="ExternalInput")
    t_dinv = nc.dram_tensor("dinvb", [P, bpc], F32, kind="ExternalInput")
    t_idxA = nc.dram_tensor("idxA", [P, CA], I16, kind="ExternalInput")
    t_idxB = nc.dram_tensor("idxB", [P, CB], I16, kind="ExternalInput")
    t_out = nc.dram_tensor("out", [bpc * P, P], F32, kind="ExternalOutput")

    ag_in = {}
    ag_out = {}
    for lyr in (1, 2):
        ag_in[lyr, "A"] = nc.dram_tensor(f"agA{lyr}_in", [nbA * P, P], BF16,
                                         kind="Internal")
        ag_out[lyr, "A"] = nc.dram_tensor(f"agA{lyr}_out", [rowsA, P], BF16,
                                          kind="Internal", addr_space="Shared")
        if nbB > 0:
            ag_in[lyr, "B"] = nc.dram_tensor(f"agB{lyr}_in", [nbB * P, P],
                                             BF16, kind="Internal")
            ag_out[lyr, "B"] = nc.dram_tensor(f"agB{lyr}_out", [rowsB, P],
                                              BF16, kind="Internal",
                                              addr_space="Shared")

    cumKl = np.concatenate([[0], np.cumsum(Kl)]).astype(int)
    cumKh = np.concatenate([[0], np.cumsum(Kh)]).astype(int)
    ga_max = int(Kl.max())
    gb_max = int(Kh.max())
    bw_max = max(g for _, g in batches) * P
    # index of the last batch whose staging rows land in table A
    last_a_batch = max((i for i, (t0, _) in enumerate(batches) if t0 < nbA),
                      default=-1)

    with tile.TileContext(nc) as tc:
        with tc.tile_pool(name="const", bufs=1) as cpool, \
             tc.tile_pool(name="sb", bufs=2) as sb, \
             tc.tile_pool(name="gat", bufs=_GATBUFS) as gat, \
             tc.tile_pool(name="ixp", bufs=_GATBUFS) as ixp, \
             tc.tile_pool(name="ps", bufs=2, space="PSUM") as ps, \
             tc.tile_pool(name="ps2", bufs=2, space="PSUM") as ps2:

            w1_t = cpool.tile([P, P], F32)
            w2_t = cpool.tile([P, P], F32)
            need_lnp = any(flags.values())
            lnp_t = cpool.tile([P, 6 * P], F32) if need_lnp else None
            dinv_t = cpool.tile([P, bpc], F32)
            ident = cpool.tile([P, P], F32)
            nc.sync.dma_start(w1_t[:], t_w1[:])
            nc.sync.dma_start(w2_t[:], t_w2[:])
            if need_lnp:
                nc.sync.dma_start(lnp_t[:], t_lnp[:])
            nc.sync.dma_start(dinv_t[:], t_dinv[:])
            make_identity(nc, ident[:])

            def stage(lyr, t0, G):
                tab = "A" if t0 < nbA else "B"
                r0 = t0 * P if tab == "A" else (t0 - nbA) * P
                rows = ag_in[lyr, tab][r0:r0 + G * P, :]
                return rows.rearrange("(b p) d -> p b d", b=G)

            def allgather(lyr, tab):
                nc.gpsimd.collective_compute(
                    "AllGather", ALU.bypass,
                    replica_groups=[list(range(NC))],
                    ins=[ag_in[lyr, tab][:]],
                    outs=[ag_out[lyr, tab][:]],
                )

            # persistent per-layer partial sums (self-loop + table-A terms).
            # pass-B applies the dest-side dinv at the end, so the self term
            # seeded here carries only the source-side factor.
            acc = cpool.tile([P, bpc * P], BF16, name="acc")

            def conv_pass_a(t0, G, lyr, acc):
                """Per-iteration table-A gathers (exact Kl[t] slots each),
                reduced and added into the persistent accumulator columns
                (initialized with the local self-loop term in the mm phase).
                Depends on AG-A only."""
                for t in range(t0, t0 + G):
                    kl = int(Kl[t])
                    if kl == 0:
                        continue
                    w = kl * P
                    g = gat.tile([P, max(ga_max, 1) * P], BF16, tag="ga")
                    ixa = ixp.tile([P, max(ga_max, 1) * 8], I16,
                                   tag="ixa", name="ixa")[:, :kl * 8]
                    nc.sync.dma_start(
                        ixa, t_idxA[:, cumKl[t] * 8:cumKl[t + 1] * 8])
                    pieces = _pieces(kl)
                    for s0, s1 in pieces:
                        nc.gpsimd.dma_gather(
                            out_ap=g[:, s0 * P:s1 * P].rearrange(
                                "p (k d) -> p k d", d=P),
                            in_ap=ag_out[lyr, "A"][:],
                            idxs_ap=ixa[:, s0 * 8:s1 * 8],
                            num_idxs=(s1 - s0) * P, num_idxs_reg=(s1 - s0) * P,
                            elem_size=P, single_packet=False,
                            queue_num=next_q())
                    ra = sb.tile([P, P], F32, tag="ra", name="ra")
                    nc.vector.tensor_reduce(
                        out=ra[:],
                        in_=g[:, :w].rearrange("p (k d) -> p d k", d=P),
                        axis=mybir.AxisListType.X, op=ALU.add)
                    rc = sb.tile([P, P], BF16, tag="rc", name="rc")
                    nc.vector.tensor_copy(rc, ra)
                    av = acc[:, t * P:(t + 1) * P]
                    nc.vector.tensor_tensor(out=av, in0=av, in1=rc,
                                            op=ALU.add)

            def conv_pass_b(t0, G, lyr, acc):
                """Per-iteration table-B gathers, combined with the
                accumulator, then batched dest-norm + LayerNorm + ELU.
                Returns t1 = elu + 1. Depends on AG-B."""
                dv = dinv_t[:, t0:t0 + G].to_broadcast((P, G, P))
                red = sb.tile([P, bw_max], F32, tag="red", name="red")[:, :G * P]
                rv = red.rearrange("p (b d) -> p b d", b=G)
                av = acc[:, t0 * P:(t0 + G) * P].rearrange(
                    "p (b d) -> p b d", b=G)
                redb = sb.tile([P, bw_max], F32, tag="redb",
                               name="redb")[:, :G * P]
                any_b = False
                for t in range(t0, t0 + G):
                    kh = int(Kh[t])
                    col = redb[:, (t - t0) * P:(t - t0 + 1) * P]
                    if kh == 0:
                        nc.vector.memset(col, 0.0)
                        continue
                    any_b = True
                    w = kh * P
                    g = gat.tile([P, max(gb_max, 1) * P], BF16, tag="gb")
                    ixb = ixp.tile([P, max(gb_max, 1) * 8], I16,
                                   tag="ixb", name="ixb")[:, :kh * 8]
                    nc.sync.dma_start(
                        ixb, t_idxB[:, cumKh[t] * 8:cumKh[t + 1] * 8])
                    pieces = _pieces(kh)
                    for s0, s1 in pieces:
                        nc.gpsimd.dma_gather(
                            out_ap=g[:, s0 * P:s1 * P].rearrange(
                                "p (k d) -> p k d", d=P),
                            in_ap=ag_out[lyr, "B"][:],
                            idxs_ap=ixb[:, s0 * 8:s1 * 8],
                            num_idxs=(s1 - s0) * P, num_idxs_reg=(s1 - s0) * P,
                            elem_size=P, single_packet=False,
                            queue_num=next_q())
                    nc.vector.tensor_reduce(
                        out=col,
                        in_=g[:, :w].rearrange("p (k d) -> p d k", d=P),
                        axis=mybir.AxisListType.X, op=ALU.add)
                nc.vector.tensor_copy(red, acc[:, t0 * P:(t0 + G) * P])
                if any_b:
                    nc.vector.tensor_tensor(
                        out=rv, in0=rv,
                        in1=redb.rearrange("p (b d) -> p b d", b=G),
                        op=ALU.add)
                # dest-side norm
                nc.vector.tensor_tensor(out=rv, in0=rv, in1=dv,
                                        op=ALU.mult)
                brow = 0 if lyr == 1 else 3
                if flags[f"use_b{lyr}"]:
                    bc = lnp_t[:, brow * P:(brow + 1) * P].rearrange(
                        "p d -> p () d").to_broadcast((P, G, P))
                    nc.vector.tensor_tensor(out=rv, in0=rv, in1=bc, op=ALU.add)
                # LayerNorm (stats per node = per (partition, block))
                s = sb.tile([P, bw_max // P], F32, tag="s", name="s")[:, :G]
                nc.vector.tensor_reduce(out=s, in_=rv,
                                        axis=mybir.AxisListType.X, op=ALU.add)
                nmu = sb.tile([P, bw_max // P], F32, tag="nmu", name="nmu")[:, :G]
                nc.vector.tensor_scalar_mul(nmu, s, -1.0 / D)
                cen = sb.tile([P, bw_max], F32, tag="cen", name="cen")[:, :G * P]
                cv = cen.rearrange("p (b d) -> p b d", b=G)
                nc.vector.tensor_tensor(
                    out=cv, in0=rv,
                    in1=nmu.rearrange("p b -> p b ()").to_broadcast((P, G, P)),
                    op=ALU.add)
                sq = sb.tile([P, bw_max], F32, tag="sq", name="sq")[:, :G * P]
                nc.scalar.activation(out=sq, in_=cen, func=AF.Square)
                vs = sb.tile([P, bw_max // P], F32, tag="vs", name="vs")[:, :G]
                nc.vector.tensor_reduce(
                    out=vs, in_=sq.rearrange("p (b d) -> p b d", b=G),
                    axis=mybir.AxisListType.X, op=ALU.add)
                var = sb.tile([P, bw_max // P], F32, tag="var", name="var")[:, :G]
                nc.vector.tensor_scalar(out=var, in0=vs, scalar1=1.0 / D,
                                        scalar2=LN_EPS, op0=ALU.mult,
                                        op1=ALU.add)
                rec = sb.tile([P, bw_max // P], F32, tag="rec", name="rec")[:, :G]
                nc.vector.reciprocal(rec, var)
                rstd = sb.tile([P, bw_max // P], F32, tag="rstd", name="rstd")[:, :G]
                nc.scalar.activation(out=rstd, in_=rec, func=AF.Sqrt)
                y = cen
                yv = cv
                nc.vector.tensor_tensor(
                    out=yv, in0=cv,
                    in1=rstd.rearrange("p b -> p b ()").to_broadcast((P, G, P)),
                    op=ALU.mult)
                grow = 1 if lyr == 1 else 4
                berow = 2 if lyr == 1 else 5
                if flags[f"use_g{lyr}"]:
                    bc = lnp_t[:, grow * P:(grow + 1) * P].rearrange(
                        "p d -> p () d").to_broadcast((P, G, P))
                    nc.vector.tensor_tensor(out=yv, in0=yv, in1=bc,
                                            op=ALU.mult)
                if flags[f"use_be{lyr}"]:
                    bc = lnp_t[:, berow * P:(berow + 1) * P].rearrange(
                        "p d -> p () d").to_broadcast((P, G, P))
                    nc.vector.tensor_tensor(out=yv, in0=yv, in1=bc,
                                            op=ALU.add)
                # ELU + 1 = relu(y) + exp(min(y, 0))
                m = sb.tile([P, bw_max], F32, tag="m", name="m")[:, :G * P]
                nc.vector.tensor_scalar_min(m, y, 0.0)
                e = sb.tile([P, bw_max], F32, tag="e", name="e")[:, :G * P]
                nc.scalar.activation(out=e, in_=m, func=AF.Exp)
                r = sb.tile([P, bw_max], F32, tag="m", name="r")[:, :G * P]
                nc.vector.tensor_scalar_max(r, y, 0.0)
                t1 = sb.tile([P, bw_max], F32, tag="t1", name="t1")[:, :G * P]
                nc.vector.tensor_tensor(out=t1, in0=r, in1=e, op=ALU.add)
                return t1

            for _rep in range(reps):
                # ------------- phase 1: hs1 = (x @ W1) * dinv ---------------
                for i, (t0, G) in (enumerate(batches) if stop >= 1 else ()):
                    xT_b = sb.tile([P, bw_max], F32, tag="xT", name="xT")[:, :G * P]
                    nc.sync.dma_start(xT_b, t_xT[:, t0 * P:(t0 + G) * P])
                    mm = ps.tile([P, bw_max], F32, tag="mm", name="mm",
                                 space="PSUM")[:, :G * P]
                    for j in range(G):
                        nc.tensor.matmul(out=mm[:, j * P:(j + 1) * P],
                                         lhsT=xT_b[:, j * P:(j + 1) * P],
                                         rhs=w1_t[:], start=(j % 4 == 0),
                                         stop=(j % 4 == 3 or j == G - 1))
                    hs = sb.tile([P, bw_max], BF16, tag="hs",
                                 name="hs")[:, :G * P]
                    nc.vector.tensor_tensor(
                        out=hs.rearrange("p (b d) -> p b d", b=G),
                        in0=mm.rearrange("p (b d) -> p b d", b=G),
                        in1=dinv_t[:, t0:t0 + G].to_broadcast((P, G, P)),
                        op=ALU.mult)
                    # local self-loop term (source-side factor only):
                    # acc = (x@W) * dinv; pass-B multiplies by dinv[dst].
                    nc.vector.tensor_tensor(
                        out=acc[:, t0 * P:(t0 + G) * P].rearrange(
                            "p (b d) -> p b d", b=G),
                        in0=mm.rearrange("p (b d) -> p b d", b=G),
                        in1=dinv_t[:, t0:t0 + G].to_broadcast((P, G, P)),
                        op=ALU.mult)
                    nc.sync.dma_start(
                        stage(1, t0, G),
                        hs.rearrange("p (b d) -> p b d", b=G))
                    if stop >= 2 and i == last_a_batch:
                        allgather(1, "A")

                # ------------- phase 2: conv1 (A pass, then B pass) ---------
                for t0, G in (batches if stop >= 3 else ()):
                    conv_pass_a(t0, G, 1, acc)
                # issue AG-B after pass-A triggers so its collective time
                # overlaps the in-flight A gathers/reduces
                if stop >= 2 and nbB > 0:
                    allgather(1, "B")
                for i, (t0, G) in (enumerate(batches) if stop >= 3 else ()):
                    t1 = conv_pass_b(t0, G, 1, acc)
                    if stop == 3:
                        nc.sync.dma_start(
                            t_out[t0 * P:(t0 + G) * P, :].rearrange(
                                "(b p) d -> p b d", b=G),
                            t1.rearrange("p (b d) -> p b d", b=G))
                        continue
                    nc.vector.tensor_scalar_add(t1, t1, -1.0)
                    z = t1
                    nc.vector.tensor_tensor(
                        out=z.rearrange("p (b d) -> p b d", b=G),
                        in0=z.rearrange("p (b d) -> p b d", b=G),
                        in1=dinv_t[:, t0:t0 + G].to_broadcast((P, G, P)),
                        op=ALU.mult)
                    zT_ps = ps2.tile([P, bw_max], F32, tag="zT", name="zT",
                                     space="PSUM")[:, :G * P]
                    for j in range(G):
                        nc.tensor.matmul(out=zT_ps[:, j * P:(j + 1) * P],
                                         lhsT=z[:, j * P:(j + 1) * P],
                                         rhs=ident[:], is_transpose=True,
                                         start=(j % 4 == 0),
                                         stop=(j % 4 == 3 or j == G - 1))
                    zT = sb.tile([P, bw_max], F32, tag="zTs", name="zTs")[:, :G * P]
                    nc.vector.tensor_copy(zT, zT_ps)
                    mm2 = ps.tile([P, bw_max], F32, tag="mm", name="mm",
                                  space="PSUM")[:, :G * P]
                    for j in range(G):
                        nc.tensor.matmul(out=mm2[:, j * P:(j + 1) * P],
                                         lhsT=zT[:, j * P:(j + 1) * P],
                                         rhs=w2_t[:], start=(j % 4 == 0),
                                         stop=(j % 4 == 3 or j == G - 1))
                    hs2 = sb.tile([P, bw_max], BF16, tag="hs",
                                  name="hs")[:, :G * P]
                    nc.vector.tensor_copy(hs2, mm2)
                    hs2_f32 = mm2
                    # layer-2 self-loop term overwrites acc after pass-B
                    # read these columns (WAR handled by the tile deps).
                    # mm2 = dinv*h2 already carries the source-side factor;
                    # pass-B's dest-side dinv completes dinv^2 * h2.
                    nc.vector.tensor_copy(
                        acc[:, t0 * P:(t0 + G) * P], hs2_f32)
                    nc.sync.dma_start(
                        stage(2, t0, G),
                        hs2.rearrange("p (b d) -> p b d", b=G))
                    if stop >= 4 and i == last_a_batch:
                        allgather(2, "A")

                # ------------- phase 3: conv2 (A pass, then B pass) ---------
                for t0, G in (batches if stop >= 5 else ()):
                    conv_pass_a(t0, G, 2, acc)
                if stop >= 4 and nbB > 0:
                    allgather(2, "B")
                for t0, G in (batches if stop >= 5 else ()):
                    t1 = conv_pass_b(t0, G, 2, acc)
                    fin = t1
                    nc.vector.tensor_scalar_add(fin, t1, -1.0)
                    nc.sync.dma_start(
                        t_out[t0 * P:(t0 + G) * P, :].rearrange(
                            "(b p) d -> p b d", b=G),
                        fin.rearrange("p (b d) -> p b d", b=G))

    nc.compile()
    return nc


# ----------------------------------------------------------------------------
# Entry point
# ----------------------------------------------------------------------------

_CONV_UPTO = "full"
_SPLIT = 1  # split big per-iteration gathers across queues
_GATBUFS = 8  # in-flight gather tiles per table
_CAP = 176   # summed slot width per compute batch
_PIECE = 14  # 0: halve gathers >=16 slots; else target slots/piece

_CACHE = {}


_META_CACHE = {}


def _get_compiled(edge_index, n_nodes, flags, reps=1, stop=99):
    import hashlib
    h = hashlib.sha1(np.ascontiguousarray(edge_index).tobytes()).hexdigest()
    mkey = (h, n_nodes, _CAP)
    if mkey not in _META_CACHE:
        _META_CACHE[mkey] = _preprocess(edge_index, n_nodes)
    key = (h, n_nodes, tuple(sorted(flags.items())), reps, stop, _SPLIT, _GATBUFS, _CAP, _PIECE)
    if key not in _CACHE:
        meta = _META_CACHE[mkey]
        nc = _build_nc(meta, flags, reps=reps, stop=stop)
        _CACHE[key] = (meta, nc)
    return _CACHE[key]


def _in_maps(inputs, meta):
    """Per-core input dicts for the compiled NEFF."""
    x = np.asarray(inputs["x"], dtype=np.float32)
    N = x.shape[0]
    bpc = meta["bpc"]
    lnp = np.zeros((P, 6 * P), dtype=np.float32)
    for i, k in enumerate(("b1", "g1", "be1", "b2", "g2", "be2")):
        lnp[:, i * P:(i + 1) * P] = np.asarray(
            inputs[k], dtype=np.float32)[None, :]
    x_pad = np.zeros((meta["NPAD"], P), dtype=np.float32)
    x_pad[:N] = x
    in_maps = []
    for c in range(NC):
        nm = meta["node_map"][c]
        in_maps.append({
            "xT": np.ascontiguousarray(x_pad[nm].T),
            "w1": np.ascontiguousarray(inputs["W1"], dtype=np.float32),
            "w2": np.ascontiguousarray(inputs["W2"], dtype=np.float32),
            "lnp": lnp,
            "dinvb": np.ascontiguousarray(
                meta["dinv_pad"][nm].reshape(bpc, P).T),
            "idxA": np.ascontiguousarray(meta["idxA"][c]),
            "idxB": np.ascontiguousarray(meta["idxB"][c]),
        })
    return in_maps


def _run(x, edge_index, W1, b1, g1, be1, W2, b2, g2, be2,
         trace=False, reps=1, stop=99, **run_kwargs):
    x = np.asarray(x, dtype=np.float32)
    N, D = x.shape
    assert D == P
    flags = {
        "use_b1": bool(np.any(np.asarray(b1) != 0)),
        "use_g1": bool(np.any(np.asarray(g1) != 1)),
        "use_be1": bool(np.any(np.asarray(be1) != 0)),
        "use_b2": bool(np.any(np.asarray(b2) != 0)),
        "use_g2": bool(np.any(np.asarray(g2) != 1)),
        "use_be2": bool(np.any(np.asarray(be2) != 0)),
    }
    meta, nc = _get_compiled(np.asarray(edge_index), N, flags, reps=reps,
                             stop=stop)
    in_maps = _in_maps(
        dict(x=x, W1=W1, b1=b1, g1=g1, be1=be1, W2=W2, b2=b2, g2=g2,
             be2=be2), meta)

    res = bass_utils.run_bass_kernel_spmd(
        nc, in_maps, core_ids=list(range(NC)), trace=trace, **run_kwargs)

    out = np.empty((meta["NPAD"], P), dtype=np.float32)
    for c in range(NC):
        out[meta["node_map"][c]] = res.results[c]["out"]
    return out[:N], res


def kernel(**inputs):
    out, _ = _run(**inputs)
    return out

